# revision 1
# baseline (speedup 1.0000x reference)
"""Causal multi-head attention block (QKV proj -> causal MHA -> out proj) on 8 Trainium2
cores.

Sharding: core = b*2 + hh handles batch b (of 4) and head-half hh (8 of 16 heads),
computing attention for its heads over the full sequence, then a partial output
projection over its 512 y-channels for all 2048 tokens. A pairwise ReduceScatter
([0,1],[2,3],...) sums the two partials of each batch and leaves each core with its
token-half of the final output. Host-side work is pure slicing/concatenation.

Matmuls run in float32r (single-pass reduced-precision fp32 on the PE, ~1e-4 rel err);
everything else is fp32.
"""

import numpy as np

import concourse.bass as bass
import concourse.tile as tile
from concourse import bacc, mybir
from concourse.bass_utils import run_bass_kernel_spmd

F32 = mybir.dt.float32
F32R = mybir.dt.float32r
AF = mybir.ActivationFunctionType

B, T, C, H = 4, 2048, 1024, 16
D = C // H          # 64
NHL = H // 2        # 8 local heads per core
NHP = NHL // 2      # 4 local head pairs
FL = NHL * D        # 512 local features
NCC = C // 128      # 8 contraction chunks over C
NTB = T // 128      # 16 token blocks
NTT = T // 512      # 4 token tiles / qtiles
NEG = -1.0e30


def build():
    nc = bacc.Bacc("TRN2", target_bir_lowering=False, num_devices=8)

    xb = nc.dram_tensor("xb", [T, C], F32R, kind="ExternalInput")
    wq = nc.dram_tensor("wq", [C, FL], F32R, kind="ExternalInput")
    wk = nc.dram_tensor("wk", [C, FL], F32R, kind="ExternalInput")
    wv = nc.dram_tensor("wv", [C, FL], F32R, kind="ExternalInput")
    wo = nc.dram_tensor("wo", [FL, C], F32R, kind="ExternalInput")
    bq = nc.dram_tensor("bq", [FL], F32, kind="ExternalInput")
    bk = nc.dram_tensor("bk", [FL], F32, kind="ExternalInput")
    bvb = nc.dram_tensor("bvb", [128, FL], F32, kind="ExternalInput")
    bob = nc.dram_tensor("bob", [128, C], F32, kind="ExternalInput")  # bo/2 broadcast
    ident = nc.dram_tensor("ident", [128, 128], F32R, kind="ExternalInput")
    mask4 = nc.dram_tensor("mask4", [128, 2048], F32, kind="ExternalInput")
    vones = nc.dram_tensor("vones", [128, NHL], F32R, kind="ExternalInput")
    zh = nc.dram_tensor("zh", [T // 2, C], F32, kind="ExternalOutput")

    with tile.TileContext(nc) as tc:
        with (
            tc.tile_pool(name="res", bufs=1) as res,
            tc.tile_pool(name="dram", bufs=1, space="DRAM") as dram,
        ):
            # resident tensors: Q^T, K^T [128, 4hp x 2048tok]; V+ones [128, 16tb x 520]
            qt_sb = res.tile([128, NHP * T], F32R)
            kt_sb = res.tile([128, NHP * T], F32R)
            v_sb = res.tile([128, NTB * (NHL * 65)], F32R)
            zpart = dram.tile([T, C], F32)
            zreds = [
                dram.tile([128, C], F32, name=f"zred{i}") for i in range(8)
            ]

            # ---------------- phase 1: x^T and QKV projections ----------------
            with (
                tc.tile_pool(name="p1", bufs=3) as p1,
                tc.tile_pool(name="p1c", bufs=1) as p1c,
                tc.tile_pool(name="tp_ps", bufs=4, space="PSUM") as tp_ps_pool,
                tc.tile_pool(name="qkv_ps", bufs=3, space="PSUM") as qkv_ps_pool,
            ):
                id_sb = p1c.tile([128, 128], F32R, tag="ident")
                nc.sync.dma_start(id_sb[:], ident[:, :])
                # warm the exp table set during phase 1 (hides ~2.7us ACT_TABLE_LOAD)
                warm = p1c.tile([1, 1], F32, tag="warm")
                nc.scalar.activation(warm[:], id_sb[0:1, 0:1].bitcast(F32), AF.Exp)
                bq_sb = p1c.tile([128, NHP], F32, tag="bq")
                nc.sync.dma_start(bq_sb[:], bq.rearrange("(f p) -> p f", p=128))
                bk_sb = p1c.tile([128, NHP], F32, tag="bk")
                nc.sync.dma_start(bk_sb[:], bk.rearrange("(f p) -> p f", p=128))
                bvb_sb = p1c.tile([128, FL], F32, tag="bvb")
                nc.sync.dma_start(bvb_sb[:], bvb[:, :])
                wv_sb = p1c.tile([128, NCC * FL], F32R, tag="wv")
                nc.sync.dma_start(
                    wv_sb[:].rearrange("p (c f) -> p c f", c=NCC),
                    wv.rearrange("(c p) f -> p c f", p=128),
                )
                # x^T: [128, 8cc x 2048tok]
                xt = p1c.tile([128, NCC * T], F32R, tag="xt")
                for tt in range(NTT):
                    for tb in range(4 * tt, 4 * tt + 4):
                        xnat = p1.tile([128, C], F32R, tag="xnat", name=f"xnat{tb}")
                        nc.sync.dma_start(xnat[:], xb[tb * 128:(tb + 1) * 128, :])
                        for cg in range(NCC // 4):
                            tp_ps = tp_ps_pool.tile([128, 512], F32R, tag="tp",
                                                    name=f"tp{tb}_{cg}")
                            for k in range(4):
                                cc = cg * 4 + k
                                nc.tensor.transpose(
                                    tp_ps[:, k * 128:(k + 1) * 128],
                                    xnat[:, cc * 128:(cc + 1) * 128], id_sb[:]
                                )
                            nc.scalar.activation(
                                xt[:].rearrange("p (c t) -> p c t", c=NCC)[
                                    :, cg * 4:(cg + 1) * 4, tb * 128:(tb + 1) * 128
                                ],
                                tp_ps[:].rearrange("p (k t) -> p k t", k=4),
                                AF.Copy,
                            )
                    # K^T and Q^T columns for this token tile, with bias
                    for w_dram, b_sb, dst, wnm in (
                        (wk, bk_sb, kt_sb, "k"), (wq, bq_sb, qt_sb, "q")
                    ):
                        for fb in range(NHP):
                            w_t = p1.tile([128, NCC * 128], F32R, tag="wqk",
                                          name=f"w{wnm}{tt}_{fb}")
                            nc.sync.dma_start(
                                w_t[:].rearrange("p (c f) -> p c f", c=NCC),
                                w_dram[:, fb * 128:(fb + 1) * 128].rearrange(
                                    "(c p) f -> p c f", p=128
                                ),
                            )
                            ps = qkv_ps_pool.tile([128, 512], F32, tag="qkv",
                                                  name=f"ps{wnm}{tt}_{fb}")
                            for cc in range(NCC):
                                nc.tensor.matmul(
                                    ps[:],
                                    w_t[:, cc * 128:(cc + 1) * 128],
                                    xt[:, cc * T + tt * 512: cc * T + (tt + 1) * 512],
                                    start=(cc == 0),
                                    stop=(cc == NCC - 1),
                                )
                            nc.scalar.activation(
                                dst[:, fb * T + tt * 512: fb * T + (tt + 1) * 512],
                                ps[:],
                                AF.Identity,
                                bias=b_sb[:, fb:fb + 1],
                            )
                    # V rows for this token tile, with bias + ones columns
                    for tb in range(4 * tt, 4 * tt + 4):
                        ps = qkv_ps_pool.tile([128, 512], F32, tag="qkv",
                                              name=f"psv{tb}")
                        for cc in range(NCC):
                            nc.tensor.matmul(
                                ps[:],
                                xt[:, cc * T + tb * 128: cc * T + (tb + 1) * 128],
                                wv_sb[:, cc * FL:(cc + 1) * FL],
                                start=(cc == 0),
                                stop=(cc == NCC - 1),
                            )
                        vslice = v_sb[:, tb * (NHL * 65):(tb + 1) * (NHL * 65)]
                        v3 = vslice.rearrange("p (h c) -> p h c", h=NHL)
                        nc.vector.tensor_add(
                            v3[:, :, 0:D],
                            ps[:].rearrange("p (h d) -> p h d", h=NHL),
                            bvb_sb[:].rearrange("p (h d) -> p h d", h=NHL),
                        )
                        nc.sync.dma_start(v3[:, :, D:D + 1], vones[:, :].unsqueeze(2))

            # ---------------- phase 2+3: attention, out-proj ----------------
            with (
                tc.tile_pool(name="ysb_pool", bufs=1) as ysb_pool,
                tc.tile_pool(name="p2", bufs=6) as p2,
                tc.tile_pool(name="p2c", bufs=1) as p2c,
                tc.tile_pool(name="norm", bufs=3) as norm,
                tc.tile_pool(name="normd", bufs=4, space="DRAM") as normd,
                tc.tile_pool(name="s_ps", bufs=2, space="PSUM") as s_ps_pool,
                tc.tile_pool(name="yu_ps", bufs=2, space="PSUM") as yu_ps_pool,
                tc.tile_pool(name="z_ps", bufs=2, space="PSUM") as z_ps_pool,
            ):
                ysb = ysb_pool.tile([128, NHP * T], F32R)
                m4_sb = p2c.tile([128, 2048], F32, tag="mask")
                nc.sync.dma_start(m4_sb[:], mask4[:, :])

                def attention_qt(qt):
                    for hp in range(NHP):
                        n_kb = 4 * (qt + 1)
                        n_kg = n_kb // 2
                        yus = [
                            yu_ps_pool.tile([65, 512], F32, tag="yu", name=f"yu{qt}_{hp}_{i}")
                            for i in range(2)
                        ]
                        qsl = qt_sb[:, hp * T + qt * 512: hp * T + (qt + 1) * 512]
                        for kg in range(n_kg):
                            # per-kblock causal offset: c = kb - 4*qt in 0..3 on the
                            # diagonal; queries j < c*128 are fully masked -> skip
                            ss = [
                                s_ps_pool.tile([128, 1024], F32, tag="s", name=f"s{qt}_{hp}_{kg}_{i}")
                                for i in range(2)
                            ]
                            j0s = []
                            for c2 in range(2):
                                kb = kg * 2 + c2
                                c = kb - 4 * qt
                                j0s.append(c * 128 if c > 0 else 0)
                            for hi in range(2):
                                for c2 in range(2):
                                    kb = kg * 2 + c2
                                    j0 = j0s[c2]
                                    nc.tensor.matmul(
                                        ss[hi][:, c2 * 512 + j0:(c2 + 1) * 512],
                                        kt_sb[
                                            hi * 64:(hi + 1) * 64,
                                            hp * T + kb * 128: hp * T + (kb + 1) * 128,
                                        ],
                                        qsl[hi * 64:(hi + 1) * 64, j0:],
                                        tile_position=(hi * 64, 0),
                                        start=True,
                                        stop=True,
                                    )
                            for c2 in range(2):
                                kb = kg * 2 + c2
                                c = kb - 4 * qt
                                if 0 <= c <= 3:
                                    # triangular band: only cols [c*128, (c+1)*128)
                                    b0 = c2 * 512 + c * 128
                                    m0 = c * 512 + c * 128
                                    for hi in range(2):
                                        nc.vector.tensor_add(
                                            ss[hi][:, b0:b0 + 128],
                                            ss[hi][:, b0:b0 + 128],
                                            m4_sb[:, m0:m0 + 128],
                                        )
                            for hi in range(2):
                                at = p2.tile([128, 1024], F32R, tag="attn")
                                if j0s[0] >= 256:
                                    # heavily masked pair: exp only valid suffixes
                                    nc.scalar.activation(
                                        at[:, j0s[0]:512], ss[hi][:, j0s[0]:512],
                                        AF.Exp, scale=0.125,
                                    )
                                    nc.scalar.activation(
                                        at[:, 512 + j0s[1]:1024],
                                        ss[hi][:, 512 + j0s[1]:1024],
                                        AF.Exp, scale=0.125,
                                    )
                                else:
                                    nc.scalar.activation(
                                        at[:], ss[hi][:], AF.Exp, scale=0.125
                                    )
                                for c2 in range(2):
                                    kb = kg * 2 + c2
                                    j0 = j0s[c2]
                                    h = 2 * hp + hi
                                    vsl = v_sb[
                                        :,
                                        kb * (NHL * 65) + h * 65:
                                        kb * (NHL * 65) + h * 65 + 65,
                                    ]
                                    nc.tensor.matmul(
                                        yus[hi][:, j0:],
                                        vsl,
                                        at[:, c2 * 512 + j0:(c2 + 1) * 512],
                                        start=(kb == 0),
                                        stop=(kb == n_kb - 1),
                                    )
                        # normalize: y = y_u / rowsum, into ysb feature-major
                        for hi in range(2):
                            rs = norm.tile([65, 512], F32, tag="rs")
                            nc.vector.reciprocal(rs[64:65, :], yus[hi][64:65, :])
                            rs_d = normd.tile([1, 512], F32, tag="rsd",
                                              name=f"rsd{qt}_{hp}_{hi}")
                            nc.sync.dma_start(rs_d[:], rs[64:65, :])
                            bc = norm.tile([64, 512], F32, tag="bc")
                            nc.sync.dma_start(bc[:], rs_d[0:1, :].to_broadcast((64, 512)))
                            ytmp = norm.tile([64, 512], F32R, tag="ytmp")
                            nc.vector.tensor_mul(ytmp[:], yus[hi][0:64, :], bc[:])
                            nc.sync.dma_start(
                                ysb[
                                    hi * 64:(hi + 1) * 64,
                                    hp * T + qt * 512: hp * T + (qt + 1) * 512,
                                ],
                                ytmp[:],
                            )

                # partial out-projection over my 512 channels.
                # zpart rows are chunk-major: [qt0 | qt2 | qt1 | qt3] so each
                # pairwise ReduceScatter chunk is a contiguous 1024-row block.
                # chunk c holds tb c (rank0 tokens) then tb 8+c (rank1 tokens)
                ZROW = {}
                for c in range(8):
                    ZROW[c] = c * 256
                    ZROW[8 + c] = c * 256 + 128

                with (
                    tc.tile_pool(name="p3c", bufs=1) as p3c,
                    tc.tile_pool(name="p3", bufs=3) as p3,
                ):
                    wo_sb = p3c.tile([128, NHP * C], F32R, tag="wo")
                    nc.sync.dma_start(
                        wo_sb[:].rearrange("p (c n) -> p c n", c=NHP),
                        wo.rearrange("(c p) n -> p c n", p=128),
                    )
                    bob_sb = p3c.tile([128, C], F32, tag="bob")
                    nc.sync.dma_start(bob_sb[:], bob[:, :])

                    def out_proj(tbs):
                        for tb in tbs:
                            zrow = ZROW[tb]
                            for ct in range(2):
                                zps = z_ps_pool.tile(
                                    [128, 512], F32, tag="z", name=f"z{tb}_{ct}"
                                )
                                for cc in range(NHP):
                                    nc.tensor.matmul(
                                        zps[:],
                                        ysb[:, cc * T + tb * 128: cc * T + (tb + 1) * 128],
                                        wo_sb[:, cc * C + ct * 512: cc * C + (ct + 1) * 512],
                                        start=(cc == 0),
                                        stop=(cc == NHP - 1),
                                    )
                                z_sb = p3.tile(
                                    [128, 512], F32, tag="zsb", name=f"zsb{tb}_{ct}"
                                )
                                nc.vector.tensor_add(
                                    z_sb[:], zps[:], bob_sb[:, ct * 512:(ct + 1) * 512]
                                )
                                nc.sync.dma_start(
                                    zpart[zrow:zrow + 128, ct * 512:(ct + 1) * 512],
                                    z_sb[:],
                                )

                    def rs_chunk(c):
                        nc.gpsimd.collective_compute(
                            "ReduceScatter",
                            mybir.AluOpType.add,
                            replica_groups=[[0, 1], [2, 3], [4, 5], [6, 7]],
                            ins=[zpart[c * 256:(c + 1) * 256, :].opt()],
                            outs=[zreds[c].opt()],
                        )
                        nc.sync.dma_start(
                            zh[c * 128:(c + 1) * 128, :], zreds[c][:]
                        )

                    attention_qt(0)
                    attention_qt(2)
                    for c in range(4):
                        out_proj([c, 8 + c])
                        rs_chunk(c)
                    attention_qt(1)
                    attention_qt(3)
                    for c in range(4, 8):
                        out_proj([c, 8 + c])
                        rs_chunk(c)

    nc.compile()
    return nc


_NC_CACHE = None


def _get_nc():
    global _NC_CACHE
    if _NC_CACHE is None:
        _NC_CACHE = build()
    return _NC_CACHE


def _in_maps(x, Wqkv, bqkv, Wo, bo):
    x = np.ascontiguousarray(np.asarray(x, dtype=np.float32))
    Wqkv = np.ascontiguousarray(np.asarray(Wqkv, dtype=np.float32))
    bqkv = np.asarray(bqkv, dtype=np.float32)
    Wo = np.ascontiguousarray(np.asarray(Wo, dtype=np.float32))
    bo = np.asarray(bo, dtype=np.float32)

    ident = np.eye(128, dtype=np.float32)
    i_ = np.arange(128, dtype=np.int64)[:, None]
    j_ = np.arange(512, dtype=np.int64)[None, :]
    mask4 = np.concatenate(
        [np.where(i_ + c * 128 > j_, np.float32(NEG), np.float32(0.0)) for c in range(4)],
        axis=1,
    ).astype(np.float32)

    in_maps = []
    for core in range(8):
        b, hh = core // 2, core % 2
        sl = slice(hh * FL, (hh + 1) * FL)
        bv_loc = bqkv[2 * C:][sl]
        in_maps.append({
            "xb": x[b],
            "wq": np.ascontiguousarray(Wqkv[:, 0 * C:1 * C][:, sl]),
            "wk": np.ascontiguousarray(Wqkv[:, 1 * C:2 * C][:, sl]),
            "wv": np.ascontiguousarray(Wqkv[:, 2 * C:3 * C][:, sl]),
            "wo": np.ascontiguousarray(Wo[sl, :]),
            "bq": np.ascontiguousarray(bqkv[0 * C:1 * C][sl]),
            "bk": np.ascontiguousarray(bqkv[1 * C:2 * C][sl]),
            "bvb": np.broadcast_to(bv_loc[None, :], (128, FL)).copy(),
            "bob": np.broadcast_to((bo * 0.5)[None, :], (128, C)).copy(),
            "ident": ident,
            "vones": np.ones((128, NHL), dtype=np.float32),
            "mask4": mask4,
        })

    return in_maps


def _assemble(res):
    out = np.empty((B, T, C), dtype=np.float32)
    for b in range(B):
        out[b, : T // 2] = res.results[2 * b]["zh"]
        out[b, T // 2:] = res.results[2 * b + 1]["zh"]
    return out


def kernel(x, Wqkv, bqkv, Wo, bo):
    in_maps = _in_maps(x, Wqkv, bqkv, Wo, bo)
    res = run_bass_kernel_spmd(_get_nc(), in_maps, core_ids=list(range(8)))
    return _assemble(res)


def run_traced(x, Wqkv, bqkv, Wo, bo, trace_cores=None):
    in_maps = _in_maps(x, Wqkv, bqkv, Wo, bo)
    res = run_bass_kernel_spmd(
        _get_nc(), in_maps, core_ids=list(range(8)), trace=True,
        trace_cores=trace_cores,
    )
    return res



# revision 3
# speedup vs baseline: 1.1393x; 1.1393x over previous
"""Causal MHA block (QKV proj -> causal MHA -> out proj) on 8 Trainium2 cores.

Sharding: core = b*2 + hh handles batch b and head-half hh (8 of 16 heads).
Single fused pipeline over query tiles qt=0..3: QKV projection for tile qt+1
and out-projection for tile qt-1 are interleaved as PE "filler" work inside
attention(qt), so the tensor engine never waits on softmax (Act engine).
Pairwise ReduceScatter per query tile leaves each core with a 256-token
quarter of its batch's output rows.

Matmuls in float32r; softmax exp on Act; elementwise on DVE.
"""

import ml_dtypes
import numpy as np

import concourse.bass as bass
import concourse.tile as tile
from concourse import bacc, mybir
from concourse.bass_utils import run_bass_kernel_spmd

F32 = mybir.dt.float32
F32R = mybir.dt.float32r
BF16 = mybir.dt.bfloat16
AF = mybir.ActivationFunctionType

B, T, C, H = 4, 2048, 1024, 16
D = C // H          # 64
NHL = H // 2        # 8 local heads
NHP = NHL // 2      # 4 local head pairs
FL = NHL * D        # 512 local features
NCC = C // 128      # 8 contraction chunks
NEG = -1.0e30

# filler cost estimates (ns of PE time)
NS_TP = 340
NS_QK = 1750
NS_V = 1750
NS_Z = 1800

# per-qt filler emission budgets (ns per kb step), tuned via TimelineSim
BUDGETS = (1900, 750, 360, 340)


class FQ:
    """In-order filler queue with DMA pre-issue lookahead."""

    def __init__(self):
        self.items = []
        self.ri = 0
        self.pi = 0

    def add(self, ns, run, pre=None):
        self.items.append((ns, run, pre))

    def _pre(self, upto):
        while self.pi < min(upto, len(self.items)):
            p = self.items[self.pi][2]
            if p is not None:
                p()
            self.pi += 1

    def emit(self, budget):
        while self.ri < len(self.items) and budget > 0:
            self._pre(self.ri + 4)
            ns, run, _ = self.items[self.ri]
            run()
            budget -= ns
            self.ri += 1

    def emit_until(self, n):
        while self.ri < min(n, len(self.items)):
            self._pre(self.ri + 4)
            ns, run, _ = self.items[self.ri]
            run()
            self.ri += 1

    def drain(self):
        self.emit(float("inf"))


def build():
    nc = bacc.Bacc("TRN2", target_bir_lowering=False, num_devices=8)

    xb = nc.dram_tensor("xb", [T, C], F32R, kind="ExternalInput")
    wq = nc.dram_tensor("wq", [C, FL], F32R, kind="ExternalInput")
    wk = nc.dram_tensor("wk", [C, FL], F32R, kind="ExternalInput")
    wv = nc.dram_tensor("wv", [C, FL], F32R, kind="ExternalInput")
    wo = nc.dram_tensor("wo", [FL, C], F32R, kind="ExternalInput")
    bq = nc.dram_tensor("bq", [FL], F32, kind="ExternalInput")
    bk = nc.dram_tensor("bk", [FL], F32, kind="ExternalInput")
    bvb = nc.dram_tensor("bvb", [128, FL], F32, kind="ExternalInput")
    bob = nc.dram_tensor("bob", [128, C], F32, kind="ExternalInput")  # bo/2
    ident = nc.dram_tensor("ident", [128, 128], F32R, kind="ExternalInput")
    maskd = nc.dram_tensor("maskd", [128, 128], F32, kind="ExternalInput")
    vones = nc.dram_tensor("vones", [128, 2 * NHL], BF16, kind="ExternalInput")
    zh = nc.dram_tensor("zh", [T // 2, C], F32, kind="ExternalOutput")

    with tile.TileContext(nc) as tc:
        with (
            tc.tile_pool(name="res", bufs=1) as res,
            tc.tile_pool(name="dram", bufs=1, space="DRAM") as dram,
            tc.tile_pool(name="xtp", bufs=1) as xtp,
            tc.tile_pool(name="qtlp", bufs=2) as qtlp,
            tc.tile_pool(name="ysbp", bufs=3) as ysbp,
            tc.tile_pool(name="atp", bufs=4) as atp,
            tc.tile_pool(name="wp", bufs=3) as wp,
            tc.tile_pool(name="xnp", bufs=2) as xnp,
            tc.tile_pool(name="yup", bufs=1) as yup,
            tc.tile_pool(name="bcp", bufs=1) as bcp,
            tc.tile_pool(name="ystp", bufs=1) as ystp,
            tc.tile_pool(name="zsbp", bufs=2) as zsbp,
            tc.tile_pool(name="sps", bufs=2, space="PSUM") as sps,
            tc.tile_pool(name="yups", bufs=1, space="PSUM") as yups,
            tc.tile_pool(name="filps", bufs=2, space="PSUM") as filps,
        ):
            # ---------------- resident tensors ----------------
            kt_sb = res.tile([128, NHP * T], F32R)          # 32 KB/part
            v_sb = res.tile([128, 16 * (NHL * 66)], BF16)   # 16.9 KB
            wv_sb = res.tile([128, NCC * FL], F32R)         # 16 KB
            wo_sb = res.tile([128, NHP * C], F32R)          # 16 KB
            m_sb = res.tile([128, 128], F32)    # causal triangle band
            id_sb = res.tile([128, 128], F32R)
            bq_sb = res.tile([128, NHP], F32)
            bk_sb = res.tile([128, NHP], F32)
            bvb_sb = res.tile([128, FL], F32)
            bob_sb = res.tile([128, C], F32)

            nc.sync.dma_start(id_sb[:], ident[:, :])
            # warm the exp table early (hides ACT_TABLE_LOAD)
            warm = res.tile([1, 1], F32)
            nc.scalar.activation(warm[:], id_sb[0:1, 0:1].bitcast(F32), AF.Exp)

            def load_smalls():
                nc.sync.dma_start(m_sb[:], maskd[:, :])
                nc.sync.dma_start(bq_sb[:],
                                  bq.rearrange("(f p) -> p f", p=128))
                nc.sync.dma_start(bk_sb[:],
                                  bk.rearrange("(f p) -> p f", p=128))


            def load_wv():
                nc.sync.dma_start(bvb_sb[:], bvb[:, :])
                nc.sync.dma_start(
                    wv_sb[:].rearrange("p (c f) -> p c f", c=NCC),
                    wv.rearrange("(c p) f -> p c f", p=128),
                )

            def load_wo():
                nc.sync.dma_start(bob_sb[:], bob[:, :])
                nc.sync.dma_start(
                    wo_sb[:].rearrange("p (c n) -> p c n", c=NHP),
                    wo.rearrange("(c p) n -> p c n", p=128),
                )

            zparts = [dram.tile([512, C], F32, name=f"zpart{i}") for i in range(4)]
            zreds = [dram.tile([256, C], F32, name=f"zred{i}") for i in range(4)]
            rds = [dram.tile([1, 1024], F32, name=f"rd{i}") for i in range(16)]

            # per-qt big tiles (ring via pools)
            xts = {}
            qtls = {}
            ysbs = {}

            def new_qt_tiles(qt):
                xts[qt] = xtp.tile([128, NCC * 512], F32R, tag="xt",
                                   name=f"xt{qt}")
                qtls[qt] = qtlp.tile([128, NHP * 512], F32R, tag="qtl",
                                     name=f"qtl{qt}")
                ysbs[qt] = ysbp.tile([128, NHP * 512], F32R, tag="ysb",
                                     name=f"ysb{qt}")

            # ---------------- filler builders ----------------
            def mk_xnat_pre(qt, tb):
                def pre():
                    xn = xnp.tile([128, C], F32R, tag="xn", name=f"xn{tb}")
                    # two half-loads so tp(tb, 0) only waits on cols 0:512
                    nc.sync.dma_start(xn[:, 0:512],
                                      xb[tb * 128:(tb + 1) * 128, 0:512])
                    nc.sync.dma_start(xn[:, 512:C],
                                      xb[tb * 128:(tb + 1) * 128, 512:C])
                    mk_xnat_pre.cur[tb] = xn
                return pre
            mk_xnat_pre.cur = {}

            def mk_tp(qt, tb, cg, on_act=False):
                tl = tb - 4 * qt

                def run():
                    xn = mk_xnat_pre.cur[tb]
                    ps = filps.tile([128, 512], F32R, tag="fil",
                                    name=f"tp{tb}_{cg}")
                    for k in range(4):
                        cc = cg * 4 + k
                        nc.tensor.transpose(
                            ps[:, k * 128:(k + 1) * 128],
                            xn[:, cc * 128:(cc + 1) * 128], id_sb[:],
                        )
                    dst = xts[qt][:].rearrange("p (c t) -> p c t", c=NCC)[
                        :, cg * 4:(cg + 1) * 4, tl * 128:(tl + 1) * 128
                    ]
                    src = ps[:].rearrange("p (k t) -> p k t", k=4)
                    if on_act:
                        nc.scalar.activation(dst, src, AF.Copy)
                    else:
                        nc.vector.tensor_copy(dst, src)
                return run

            def mk_qk(qt, which, fb, on_act=False):
                w_dram = wk if which == "k" else wq
                b_sb = bk_sb if which == "k" else bq_sb
                holder = {}

                def pre():
                    w_t = wp.tile([128, NCC * 128], F32R, tag="w",
                                  name=f"w{which}{qt}_{fb}")
                    nc.sync.dma_start(
                        w_t[:].rearrange("p (c f) -> p c f", c=NCC),
                        w_dram[:, fb * 128:(fb + 1) * 128].rearrange(
                            "(c p) f -> p c f", p=128),
                    )
                    holder["w"] = w_t

                def run():
                    ps = filps.tile([128, 512], F32, tag="fil",
                                    name=f"ps{which}{qt}_{fb}")
                    w_t = holder["w"]
                    for cc in range(NCC):
                        nc.tensor.matmul(
                            ps[:],
                            w_t[:, cc * 128:(cc + 1) * 128],
                            xts[qt][:, cc * 512:(cc + 1) * 512],
                            start=(cc == 0), stop=(cc == NCC - 1),
                        )
                    if which == "k":
                        dst = kt_sb[:, fb * T + qt * 512: fb * T + (qt + 1) * 512]
                    else:
                        dst = qtls[qt][:, fb * 512:(fb + 1) * 512]
                    if on_act:
                        nc.scalar.activation(dst, ps[:], AF.Identity,
                                             bias=b_sb[:, fb:fb + 1])
                    else:
                        nc.vector.tensor_scalar_add(dst, ps[:],
                                                    b_sb[:, fb:fb + 1])
                return run, pre

            def mk_v(qt, tb):
                tl = tb - 4 * qt

                def run():
                    ps = filps.tile([128, 512], F32, tag="fil", name=f"psv{tb}")
                    for cc in range(NCC):
                        nc.tensor.matmul(
                            ps[:],
                            xts[qt][:, cc * 512 + tl * 128: cc * 512 + (tl + 1) * 128],
                            wv_sb[:, cc * FL:(cc + 1) * FL],
                            start=(cc == 0), stop=(cc == NCC - 1),
                        )
                    v3 = v_sb[:, tb * (NHL * 66):(tb + 1) * (NHL * 66)] \
                        .rearrange("p (h c) -> p h c", h=NHL)
                    nc.vector.tensor_add(
                        v3[:, :, 0:D],
                        ps[:].rearrange("p (h d) -> p h d", h=NHL),
                        bvb_sb[:].rearrange("p (h d) -> p h d", h=NHL),
                    )
                    nc.sync.dma_start(v3[:, :, D:D + 2],
                                      vones.rearrange("p (h c) -> p h c", c=2))
                return run

            def mk_z(qt, tb):
                tl = tb - 4 * qt

                def run():
                    for ct in range(2):
                        zp = filps.tile([128, 512], F32, tag="fil",
                                        name=f"z{tb}_{ct}")
                        for cc in range(NHP):
                            nc.tensor.matmul(
                                zp[:],
                                ysbs[qt][:, cc * 512 + tl * 128:
                                         cc * 512 + (tl + 1) * 128],
                                wo_sb[:, cc * C + ct * 512: cc * C + (ct + 1) * 512],
                                start=(cc == 0), stop=(cc == NHP - 1),
                            )
                        z_sb = zsbp.tile([128, 512], F32, tag="zsb",
                                         name=f"zsb{tb}_{ct}")
                        nc.vector.tensor_add(
                            z_sb[:], zp[:], bob_sb[:, ct * 512:(ct + 1) * 512])
                        nc.sync.dma_start(
                            zparts[qt][tl * 128:(tl + 1) * 128,
                                       ct * 512:(ct + 1) * 512],
                            z_sb[:])
                return run

            def mk_rs(qt):
                def run():
                    nc.gpsimd.collective_compute(
                        "ReduceScatter",
                        mybir.AluOpType.add,
                        replica_groups=[[0, 1], [2, 3], [4, 5], [6, 7]],
                        ins=[zparts[qt].opt()],
                        outs=[zreds[qt].opt()],
                    )
                    nc.sync.dma_start(
                        zh[qt * 256:(qt + 1) * 256, :], zreds[qt][:])
                return run

            def add_phase1(fq, qt, on_act=False):
                # tp/V interleaved so xnat buffers get recycle slack; V(tb)
                # only reads its own tb's xt columns so it can follow its tps
                new_qt_tiles(qt)
                tbs = list(range(4 * qt, 4 * qt + 4))
                fq.add(NS_TP, mk_tp(qt, tbs[0], 0, on_act),
                       mk_xnat_pre(qt, tbs[0]))
                fq.add(NS_TP, mk_tp(qt, tbs[0], 1, on_act))
                for i, tb in enumerate(tbs[1:]):
                    fq.add(NS_TP, mk_tp(qt, tb, 0, on_act),
                           mk_xnat_pre(qt, tb))
                    fq.add(NS_TP, mk_tp(qt, tb, 1, on_act))
                    fq.add(NS_V, mk_v(qt, tbs[i]))
                fq.add(NS_V, mk_v(qt, tbs[3]))
                for fb in range(NHP):
                    run, pre = mk_qk(qt, "k", fb, on_act)
                    fq.add(NS_QK, run, pre)
                for fb in range(NHP):
                    run, pre = mk_qk(qt, "q", fb, on_act)
                    fq.add(NS_QK, run, pre)

            def add_qk_only(fq, qt, which):
                for fb in range(NHP):
                    run, pre = mk_qk(qt, which, fb)
                    fq.add(NS_QK, run, pre)

            def add_v_only(fq, qt):
                for tb in range(4 * qt, 4 * qt + 4):
                    fq.add(NS_V, mk_v(qt, tb))

            def add_outproj(fq, qt):
                for tb in range(4 * qt, 4 * qt + 4):
                    fq.add(NS_Z, mk_z(qt, tb))
                fq.add(0, mk_rs(qt))

            # ---------------- attention ----------------
            def attention(qt, fq, budget, force_fn=None):
                n_kb = 4 * (qt + 1)
                for hp in range(NHP):
                    yu = yups.tile([66, 1024], F32, tag="yu")
                    qsl = qtls[qt]
                    pend = []

                    def emit_av(kb, j0, at):
                        for hi in range(2):
                            h = 2 * hp + hi
                            vsl = v_sb[:, kb * (NHL * 66) + h * 66:
                                       kb * (NHL * 66) + h * 66 + 66]
                            nc.tensor.matmul(
                                yu[:, hi * 512 + j0:(hi + 1) * 512],
                                vsl,
                                at[:, hi * 512 + j0:(hi + 1) * 512],
                                start=(kb == 0), stop=(kb == n_kb - 1),
                            )

                    for kb in range(n_kb):
                        if force_fn is not None:
                            fq.emit_until(force_fn(hp, kb))
                        c = kb - 4 * qt
                        # scores matmul start col (clamped at 256 to dodge
                        # the fp32r ap<256 penalty); mask/exp/attnV use the
                        # true causal start col
                        j0 = min(c, 2) * 128 if c > 0 else 0
                        j0x = c * 128 if c > 0 else 0
                        diag = 0 <= c <= 3
                        ss = sps.tile([128, 1024], F32, tag="ss",
                                      name=f"ss{qt}_{hp}_{kb}")
                        ss3 = ss[:].rearrange("p (h q) -> p h q", h=2)
                        for hi in range(2):
                            nc.tensor.matmul(
                                ss[:, hi * 512 + j0:(hi + 1) * 512],
                                kt_sb[hi * 64:(hi + 1) * 64,
                                      hp * T + kb * 128: hp * T + (kb + 1) * 128],
                                qsl[hi * 64:(hi + 1) * 64,
                                    hp * 512 + j0: (hp + 1) * 512],
                                tile_position=(hi * 64, 0),
                                start=True, stop=True,
                            )
                        if diag:
                            nc.vector.tensor_add(
                                ss3[:, :, j0x:j0x + 128],
                                ss3[:, :, j0x:j0x + 128],
                                m_sb[:, :].unsqueeze(1)
                                .to_broadcast((128, 2, 128)),
                            )
                        at = atp.tile([128, 1024], BF16, tag="at")
                        at3 = at[:].rearrange("p (h q) -> p h q", h=2)
                        if j0x:
                            nc.scalar.activation(
                                at3[:, :, j0x:], ss3[:, :, j0x:],
                                AF.Exp, scale=0.125)
                        else:
                            nc.scalar.activation(
                                at[:], ss[:], AF.Exp, scale=0.125)
                        pend.append((kb, j0x, at))
                        if kb >= 1:
                            fq.emit(budget)
                            emit_av(*pend.pop(0))
                    fq.emit(budget)
                    emit_av(*pend.pop(0))

                    # normalization: recip of rowsum + evict yu to SBUF;
                    # recip row broadcast via DRAM bounce (hi1 mult first so
                    # its partition-shift DMA starts earliest)
                    yusb = yup.tile([65, 1024], F32, tag="yusb")
                    nc.vector.reciprocal(yusb[64:65, :], yu[64:65, :])
                    nc.vector.tensor_copy(yusb[0:64, :], yu[0:64, :])
                    rd = rds[qt * 4 + hp]
                    nc.sync.dma_start(rd[:], yusb[64:65, :])
                    bc = bcp.tile([64, 1024], F32, tag="bc")
                    nc.sync.dma_start(bc[:],
                                      rd[0:1, :].to_broadcast((64, 1024)))
                    yst = ystp.tile([64, 512], F32R, tag="yst")
                    nc.vector.tensor_mul(yst[:], yusb[0:64, 512:1024],
                                         bc[:, 512:1024])
                    nc.sync.dma_start(
                        ysbs[qt][64:128, hp * 512:(hp + 1) * 512], yst[:])
                    nc.vector.tensor_mul(
                        ysbs[qt][0:64, hp * 512:(hp + 1) * 512],
                        yusb[0:64, 0:512], bc[:, 0:512])

            # ---------------- schedule ----------------
            # lead-in: xt(0) transposes + K/Q for head-pair 0 only
            fq0 = FQ()
            new_qt_tiles(0)
            for tb in range(4):
                fq0.add(NS_TP, mk_tp(0, tb, 0, on_act=True),
                        mk_xnat_pre(0, tb))
                fq0.add(NS_TP, mk_tp(0, tb, 1, on_act=True))
            runk, prek = mk_qk(0, "k", 0, on_act=True)
            fq0.add(NS_QK, runk, prek)
            runq, preq = mk_qk(0, "q", 0, on_act=True)
            fq0.add(NS_QK, runq, preq)
            fq0.add(0, load_wv)
            fq0._pre(2)          # first two xnat DMAs ahead of small loads
            load_smalls()
            fq0.drain()

            fq = FQ()
            add_v_only(fq, 0)                    # items 0-3: V(0)
            for fb in range(1, NHP):             # items 4-9: K/Q pairs
                runk, prek = mk_qk(0, "k", fb, on_act=True)
                fq.add(NS_QK, runk, prek)
                runq, preq = mk_qk(0, "q", fb, on_act=True)
                fq.add(NS_QK, runq, preq)
            add_phase1(fq, 1, on_act=True)
            fq.add(0, load_wo)

            def force0(hp, kb):
                if hp == 0:
                    return min(kb + 1, 4)
                return 2 * hp + 4 if kb == 0 else 0

            attention(0, fq, BUDGETS[0], force_fn=force0)
            fq.drain()

            fq = FQ()
            add_phase1(fq, 2, on_act=True)
            attention(1, fq, BUDGETS[1])
            fq.drain()

            fq = FQ()
            new_qt_tiles(3)
            for tb in range(12, 16):
                fq.add(NS_TP, mk_tp(3, tb, 0), mk_xnat_pre(3, tb))
                fq.add(NS_TP, mk_tp(3, tb, 1))
            add_qk_only(fq, 3, "q")
            add_outproj(fq, 0)
            attention(2, fq, BUDGETS[2])
            fq.drain()

            fq = FQ()
            runk0, prek0 = mk_qk(3, "k", 0)
            fq.add(NS_QK, runk0, prek0)          # item 0: K(3,0)
            add_v_only(fq, 3)                    # items 1-4: V(3,tb12..15)
            for fb in range(1, NHP):             # items 5-7: K(3,1..3)
                runk, prek = mk_qk(3, "k", fb)
                fq.add(NS_QK, runk, prek)
            add_outproj(fq, 1)

            def force3(hp, kb):
                if hp == 0:
                    return (kb + 1) * 5 // 12 if kb < 12 else kb - 10
                if kb == 12:
                    return 5 + hp
                return 0

            attention(3, fq, BUDGETS[3], force_fn=force3)
            fq.drain()

            fq = FQ()
            add_outproj(fq, 2)
            add_outproj(fq, 3)
            fq.drain()

    nc.compile()
    return nc


_NC_CACHE = None


def _get_nc():
    global _NC_CACHE
    if _NC_CACHE is None:
        _NC_CACHE = build()
    return _NC_CACHE


def _in_maps(x, Wqkv, bqkv, Wo, bo):
    x = np.ascontiguousarray(np.asarray(x, dtype=np.float32))
    Wqkv = np.ascontiguousarray(np.asarray(Wqkv, dtype=np.float32))
    bqkv = np.asarray(bqkv, dtype=np.float32)
    Wo = np.ascontiguousarray(np.asarray(Wo, dtype=np.float32))
    bo = np.asarray(bo, dtype=np.float32)

    ident = np.eye(128, dtype=np.float32)
    i_ = np.arange(128, dtype=np.int64)[:, None]
    j_ = np.arange(128, dtype=np.int64)[None, :]
    tri = np.where(i_ > j_, np.float32(NEG), np.float32(0.0))
    maskd = tri

    in_maps = []
    for core in range(8):
        b, hh = core // 2, core % 2
        sl = slice(hh * FL, (hh + 1) * FL)
        bv_loc = bqkv[2 * C:][sl]
        in_maps.append({
            "xb": x[b],
            "wq": np.ascontiguousarray(Wqkv[:, 0 * C:1 * C][:, sl]),
            "wk": np.ascontiguousarray(Wqkv[:, 1 * C:2 * C][:, sl]),
            "wv": np.ascontiguousarray(Wqkv[:, 2 * C:3 * C][:, sl]),
            "wo": np.ascontiguousarray(Wo[sl, :]),
            "bq": np.ascontiguousarray(bqkv[0 * C:1 * C][sl]),
            "bk": np.ascontiguousarray(bqkv[1 * C:2 * C][sl]),
            "bvb": np.broadcast_to(bv_loc[None, :], (128, FL)).copy(),
            "bob": np.broadcast_to((bo * 0.5)[None, :], (128, C)).copy(),
            "ident": ident,
            "maskd": maskd,
            "vones": np.ones((128, 2 * NHL), dtype=ml_dtypes.bfloat16),
        })
    return in_maps


def _assemble(res):
    out = np.empty((B, T, C), dtype=np.float32)
    for b in range(B):
        for hh in range(2):
            zc = res.results[2 * b + hh]["zh"]
            for qt in range(4):
                out[b, 512 * qt + 256 * hh: 512 * qt + 256 * hh + 256] = \
                    zc[256 * qt: 256 * qt + 256]
    return out


def kernel(x, Wqkv, bqkv, Wo, bo):
    in_maps = _in_maps(x, Wqkv, bqkv, Wo, bo)
    res = run_bass_kernel_spmd(_get_nc(), in_maps, core_ids=list(range(8)))
    return _assemble(res)


def run_traced(x, Wqkv, bqkv, Wo, bo, trace_cores=None):
    in_maps = _in_maps(x, Wqkv, bqkv, Wo, bo)
    res = run_bass_kernel_spmd(
        _get_nc(), in_maps, core_ids=list(range(8)), trace=True,
        trace_cores=trace_cores,
    )
    return res
